# revision 24
# baseline (speedup 1.0000x reference)
# Trainium2 Bass kernel for nn_AllAtomAtomAttention (sparse edge attention).
#
# Sharding: atoms (512) are split contiguously over 8 cores (64 each); edges
# are routed to the core owning their source atom (segment softmax/sum stay
# device-local), padded to EP per core. MLP weights are replicated.
#
# Device algorithm (per core), all feature-major (features on partitions):
#   - rbf/cutoff from dist on ACT; kv-MLPs on PE (bf16), full-width silus
#   - q-MLP layer 1 factorized: hq = h_loc @ Wq1a expanded to edges via a
#     one-hot matmul; e_feat part enters as the per-partition ACT bias of the
#     layer-1 silu (n-major passes)
#   - scores accumulate in PSUM via per-n block-selector stationaries (B_n)
#     applied to the elementwise q*k product (DVE); softmax max-subtraction is
#     skipped (mathematically invariant; scores are O(10)); final-layer q bias
#     enters via a separate k-contraction (corr) matmul; SCORE_SCALE is
#     applied in fp32 via the Exp activation's scale immediate
#   - exp on ACT; segment sums (denominator and attn-weighted values) as
#     matmuls against per-edge one-hot matrices; normalization after the sum
#   - out-proj MLP on PE; host reassembles/transposes the output
#
# Masked/padded edges have all-zero one-hot columns, so they drop out of every
# segment sum exactly.

import numpy as np
import ml_dtypes

import concourse.bass as bass
import concourse.mybir as mybir
import concourse.tile as tile

F32 = mybir.dt.float32
BF16 = mybir.dt.bfloat16
AF = mybir.ActivationFunctionType
ALU = mybir.AluOpType

ATT_CUTOFF = 5.0
RBF_DIM = 16
N_HEADS = 4
LATENT = 128
HEAD_DIM = 32
ATOM_DIM = 128
E_DIM = 16
Z_EMB_DIM = 32
SCORE_SCALE = HEAD_DIM ** (-0.5)

NCORES = 8
LA = 64            # atoms per core
EP = 1152          # padded edges per core
NCH = EP // 128    # 128-edge chunks
NE = 32            # edge-feature contexts
SLICES = [(0, 512), (512, 512), (1024, 128)]  # EP split into <=512 runs
OUT_COLS = NE * LA  # 2048 tokens per core in out-proj
KVIN = Z_EMB_DIM + 1  # 33

# Packed bf16 constants: (name, partitions, cols)
CBF_ENTRIES = [
    ("w_k1a", 128, 128), ("w_k1b", KVIN, 128), ("w_k1br", RBF_DIM, 128),
    ("w_k2", 128, 128), ("w_k3", 128, 128),
    ("w_v1a", 128, 128), ("w_v1b", KVIN, 128), ("w_v1br", RBF_DIM, 128),
    ("w_v2", 128, 128), ("w_v3", 128, 128),
    ("w_q1a", 128, 128), ("w_q1b", E_DIM + 1, 128),
    ("w_q2", 128, 128), ("w_q3", 128, 128),
    ("w_o1", 128, 128), ("w_o2", 128, 128),
    ("efT1", E_DIM + 1, NE),
    ("h_locT", 128, LA),          # per-core
    ("ohEc", 128, LA * NCH),      # per-core
    ("idbf", 128, 128),
]
CBF_OFF = {}
_o = 0
for _n, _p, _w in CBF_ENTRIES:
    CBF_OFF[_n] = _o
    _o += _w
CBF_W = _o

# Packed f32 constants
CF32_ENTRIES = [
    ("b_k1", 128, 1), ("b_k2", 128, 1), ("b_k3", 128, 1),
    ("b_v1", 128, 1), ("b_v2", 128, 1), ("b_v3", 128, 1),
    ("b_q2", 128, 1), ("b_o1", 128, 1), ("b_o2", 128, 1),
    ("negC", RBF_DIM, 1), ("bq3blk", 128, N_HEADS), ("C4", N_HEADS, 128),
]
CF32_OFF = {}
_o = 0
for _n, _p, _w in CF32_ENTRIES:
    CF32_OFF[_n] = _o
    _o += _w
CF32_W = _o


def _edge_slices(width):
    o = 0
    res = []
    while o < EP:
        w = min(width, EP - o)
        res.append((o, w))
        o += w
    return res


# ---------------------------------------------------------------------------
# Device program
# ---------------------------------------------------------------------------

def _split_waits(nc, max_waits=1):
    """Walrus legalization: the TRN2 codegen rejects instructions carrying
    multiple sync waits (LDWEIGHTS/table-load companions consume the slot).
    Hoist extra waits onto same-engine Drain instructions placed just before."""
    n = 0
    for fn in nc.m.functions:
        for blk in fn.blocks:
            out = []
            for inst in blk.instructions:
                si = inst.sync_info
                if si is not None and si.on_wait and len(si.on_wait) > max_waits:
                    waits = list(si.on_wait)
                    for w in waits[:-max_waits]:
                        n += 1
                        out.append(mybir.InstDrain(
                            name=f"WSPLIT-{n}",
                            engine=inst.engine,
                            ins=[], outs=[],
                            sync_info=mybir.SyncInfo(on_wait=[w], on_update=[]),
                        ))
                    inst.sync_info = mybir.SyncInfo(
                        on_wait=waits[-max_waits:], on_update=list(si.on_update))
                out.append(inst)
            blk.instructions = out
    return n


def build_nc() -> bass.Bass:
    nc = bass.Bass()

    d = {}
    def din(name, shape, dt=F32):
        d[name] = nc.dram_tensor(name, shape, dt, kind="ExternalInput")
        return d[name]

    din("d16", (RBF_DIM, EP))
    din("dE", (128, NCH))
    din("xkvbT", (KVIN, EP), BF16)
    din("hdstT", (ATOM_DIM, EP), BF16)
    din("cf32", (128, CF32_W))
    din("cbf", (128, CBF_W), BF16)
    din("ohT", (LA, EP), BF16)
    din("Bnc", (128, 128 * NE), BF16)
    out_d = nc.dram_tensor("out", (128, OUT_COLS), F32, kind="ExternalOutput")

    with tile.TileContext(nc) as tc:
        with (
            tc.tile_pool(name="const", bufs=1) as pc,
            tc.tile_pool(name="long", bufs=1) as pl,
            tc.tile_pool(name="work", bufs=3) as pw,
        ):
            # ---- inputs into SBUF (ordered: phase-A needs first) ----
            d16 = pl.tile([RBF_DIM, EP], F32, tag="d16")
            nc.sync.dma_start(d16[:], d["d16"][:])
            dE = pl.tile([128, NCH], F32, tag="dE")
            nc.sync.dma_start(dE[:], d["dE"][:])
            xkvb = pl.tile([KVIN, EP], BF16, tag="xkvb")
            nc.sync.dma_start(xkvb[:], d["xkvbT"][:])
            hdstT = pl.tile([ATOM_DIM, EP], BF16, tag="hdstT")
            nc.sync.dma_start(hdstT[:], d["hdstT"][:])
            cf32 = pc.tile([128, CF32_W], F32, tag="cf32")
            nc.sync.dma_start(cf32[:], d["cf32"][:])
            cbf = pc.tile([128, CBF_W], BF16, tag="cbf")
            nc.sync.dma_start(cbf[:], d["cbf"][:])
            ohT_s = pl.tile([LA, EP], BF16, tag="ohT")
            nc.sync.dma_start(ohT_s[:], d["ohT"][:])
            Bnc = pc.tile([128, 128 * NE], BF16, tag="Bnc")
            nc.gpsimd.dma_start(Bnc[:], d["Bnc"][:])

            def cb(name):
                _, p, w = next(e for e in CBF_ENTRIES if e[0] == name)
                return cbf[0:p, CBF_OFF[name]:CBF_OFF[name] + w]

            def cf(name):
                _, p, w = next(e for e in CF32_ENTRIES if e[0] == name)
                return cf32[0:p, CF32_OFF[name]:CF32_OFF[name] + w]

            idbf = cb("idbf")

            # long-lived activations
            kT = pl.tile([128, EP], F32, tag="kT")
            vT = pl.tile([128, EP], BF16, tag="vT")
            vE = pl.tile([128, EP], BF16, tag="vE")
            exT = pl.tile([128, EP], BF16, tag="exT")
            exE = pl.tile([128, EP], BF16, tag="exE")
            efWT_S = pl.tile([128, NE], F32, tag="efWT_S")
            corrS = pl.tile([N_HEADS, EP], F32, tag="corrS")
            rden = pl.tile([LA, NE * N_HEADS], F32, tag="rden")
            attnS = pl.tile([LA, NE * LATENT], BF16, tag="attnS")
            oT = pl.tile([128, OUT_COLS], BF16, tag="oT")
            rbft = pl.tile([RBF_DIM, EP], BF16, tag="rbft")

            centers = np.linspace(0.0, ATT_CUTOFF, RBF_DIM)
            coeff = float(-0.5 / (centers[1] - centers[0]) ** 2)

            # =========== Phase A: rbf, cutoff, kv MLPs (full-width) ========
            with tc.tile_pool(name="psA", bufs=2, space="PSUM") as ppa:
                # rbf_j = exp(coeff*(d - c_j)^2)
                t16 = pw.tile([RBF_DIM, EP], F32, tag="t16")
                nc.scalar.activation(t16[:], d16[:], AF.Square,
                                     bias=cf("negC")[:, 0:1])
                nc.scalar.activation(rbft[:], t16[:], AF.Exp, scale=coeff)

                # cutoff = 0.5*sin(pi/c*d + pi/2) + 0.5, edge-major (128, NCH)
                sinargE = pw.tile([128, NCH], F32, tag="sinargE")
                nc.vector.tensor_scalar(sinargE[:], dE[:],
                                        float(np.pi / ATT_CUTOFF),
                                        float(np.pi / 2),
                                        op0=ALU.mult, op1=ALU.add)
                cutE = pw.tile([128, NCH], F32, tag="cutE")
                nc.scalar.activation(cutE[:], sinargE[:], AF.Sin)
                nc.vector.tensor_scalar(cutE[:], cutE[:], 0.5, 0.5,
                                        op0=ALU.mult, op1=ALU.add)

                # kv MLPs, full-width psum, k/v layers interleaved
                l1s, x1s, l2s, x2s, l3s = {}, {}, {}, {}, {}
                for p in ("k", "v"):
                    l1s[p] = ppa.tile([128, EP], F32, tag="pA", name=f"l1{p}")
                    for o, w in SLICES:
                        nc.tensor.matmul(l1s[p][:, o:o + w], cb(f"w_{p}1a"),
                                         hdstT[:, o:o + w], start=True, stop=False)
                        nc.tensor.matmul(l1s[p][:, o:o + w], cb(f"w_{p}1b"),
                                         xkvb[:, o:o + w], start=False, stop=False)
                        nc.tensor.matmul(l1s[p][:, o:o + w], cb(f"w_{p}1br"),
                                         rbft[:, o:o + w], start=False, stop=True)
                for p in ("k", "v"):
                    x1s[p] = pw.tile([128, EP], BF16, tag=f"kv_x1{p}", name=f"x1{p}")
                    nc.scalar.activation(x1s[p][:], l1s[p][:], AF.Silu,
                                         bias=cf(f"b_{p}1")[:, 0:1])
                for p in ("k", "v"):
                    l2s[p] = ppa.tile([128, EP], F32, tag="pA", name=f"l2{p}")
                    for o, w in SLICES:
                        nc.tensor.matmul(l2s[p][:, o:o + w], cb(f"w_{p}2"),
                                         x1s[p][:, o:o + w], start=True, stop=True)
                for p in ("k", "v"):
                    x2s[p] = pw.tile([128, EP], BF16, tag=f"kv_x2{p}", name=f"x2{p}")
                    nc.scalar.activation(x2s[p][:], l2s[p][:], AF.Silu,
                                         bias=cf(f"b_{p}2")[:, 0:1])
                for p in ("k", "v"):
                    l3s[p] = ppa.tile([128, EP], F32, tag="pA", name=f"l3{p}")
                    for o, w in SLICES:
                        nc.tensor.matmul(l3s[p][:, o:o + w], cb(f"w_{p}3"),
                                         x2s[p][:, o:o + w], start=True, stop=True)
                nc.scalar.activation(kT[:], l3s["k"][:], AF.Identity,
                                     bias=cf("b_k3")[:, 0:1])
                nc.scalar.activation(vT[:], l3s["v"][:], AF.Identity,
                                     bias=cf("b_v3")[:, 0:1])

                # transpose v -> edge-major vE, applying cutoff as the
                # per-partition (= per-edge) scale of the PSUM->SBUF copy
                for g in range((NCH + 3) // 4):
                    tp = ppa.tile([128, 512], BF16, tag="pT")
                    nchunk = min(4, NCH - 4 * g)
                    for j in range(nchunk):
                        c = 4 * g + j
                        nc.tensor.transpose(tp[:, 128 * j:128 * (j + 1)],
                                            vT[:, 128 * c:128 * (c + 1)], idbf)
                    for j in range(nchunk):
                        c = 4 * g + j
                        nc.scalar.activation(vE[:, 128 * c:128 * (c + 1)],
                                             tp[:, 128 * j:128 * (j + 1)], AF.Copy,
                                             scale=cutE[:, c:c + 1])
                cr = ppa.tile([N_HEADS, EP], F32, tag="pA")
                for o, w in SLICES:
                    nc.tensor.matmul(cr[:, o:o + w], cf("bq3blk"),
                                     kT[:, o:o + w], start=True, stop=True)
                nc.vector.tensor_copy(corrS[:], cr[:])

                # hq path: atom-level (feature-major, 64 atom columns)
                hqp = ppa.tile([128, LA], F32, tag="pT")
                nc.tensor.matmul(hqp[:], cb("w_q1a"), cb("h_locT"),
                                 start=True, stop=True)
                hqA_S = pw.tile([128, LA], F32, tag="hqA_S")
                nc.vector.tensor_copy(hqA_S[:], hqp[:])
                efp = ppa.tile([128, NE], F32, tag="pT")
                nc.tensor.matmul(efp[:], cb("w_q1b"), cb("efT1"),
                                 start=True, stop=True)
                nc.vector.tensor_copy(efWT_S[:], efp[:])

            # ====== Phase C1: atom-level q-MLP (2048 distinct (n,atom) rows) =====
            q3ATb = pl.tile([LA, 128 * NE], BF16, tag="q3ATb")
            with (
                tc.tile_pool(name="psC1", bufs=2, space="PSUM") as pc1,
                tc.tile_pool(name="psC1b", bufs=2, space="PSUM") as pc1b,
            ):
                x1all = pw.tile([128, LA * NE], BF16, tag="x1all")
                l1all = pw.tile([128, LA * NE], F32, tag="l1all")
                a_hq = hqA_S[:].unsqueeze(1).to_broadcast((128, NE, LA))
                a_ef = efWT_S[:].unsqueeze(2).to_broadcast((128, NE, LA))
                dst1 = l1all[:].rearrange("p (n a) -> p n a", n=NE)
                nc.vector.tensor_tensor(dst1, a_ef, a_hq, op=ALU.add)
                nc.scalar.activation(x1all[:], l1all[:], AF.Silu)
                x2all = pw.tile([128, LA * NE], BF16, tag="x2all")
                for q in range(4):
                    sl = slice(512 * q, 512 * (q + 1))
                    l2 = pc1.tile([128, 512], F32, tag="q_l2")
                    nc.tensor.matmul(l2[:], cb("w_q2"), x1all[:, sl],
                                     start=True, stop=True)
                    nc.scalar.activation(x2all[:, sl], l2[:], AF.Silu,
                                         bias=cf("b_q2")[:, 0:1])
                for r in range(4):
                    q3p = pc1b.tile([LA, 1024], F32, tag="q3p")
                    for j in range(8):
                        n = 8 * r + j
                        nc.tensor.matmul(q3p[:, 128 * j:128 * (j + 1)],
                                         x2all[:, LA * n:LA * (n + 1)],
                                         cb("w_q3"), start=True, stop=True)
                    nc.scalar.activation(q3ATb[:, 1024 * r:1024 * (r + 1)],
                                         q3p[:], AF.Copy)

            # ====== Phase C2: expand to edges, scores ======
            with (
                tc.tile_pool(name="psS", bufs=1, space="PSUM") as pps,
                tc.tile_pool(name="psE", bufs=1, space="PSUM") as pse,
            ):
                scores = pps.tile([128, EP], F32, tag="scores")
                for o, w in SLICES:
                    nc.tensor.matmul(scores[:, o:o + w], cf("C4"),
                                     corrS[:, o:o + w], start=True, stop=False)
                for n in range(NE):
                    qk = pw.tile([128, EP], BF16, tag="q_qk")
                    q3E = pse.tile([128, EP], F32, tag="q3E")
                    for o, w in SLICES:
                        nc.tensor.matmul(q3E[:, o:o + w],
                                         q3ATb[:, 128 * n:128 * (n + 1)],
                                         ohT_s[:, o:o + w],
                                         start=True, stop=True)
                    nc.vector.tensor_tensor(qk[:], q3E[:], kT[:], op=ALU.mult)
                    for o, w in SLICES:
                        nc.tensor.matmul(scores[:, o:o + w],
                                         Bnc[:, 128 * n:128 * (n + 1)],
                                         qk[:, o:o + w], start=False,
                                         stop=(n == NE - 1 and o + w == EP))
                # exp (SCORE_SCALE applied here in fp32)
                nc.scalar.activation(exT[:], scores[:], AF.Exp, scale=SCORE_SCALE)

            # =========== Phase D: transposes, segment sums ===========
            with (
                tc.tile_pool(name="psT", bufs=2, space="PSUM") as ppt,
                tc.tile_pool(name="psD", bufs=1, space="PSUM") as ppd,
                tc.tile_pool(name="psN", bufs=2, space="PSUM") as ppn,
            ):
                for g in range((NCH + 3) // 4):
                    tp = ppt.tile([128, 512], BF16, tag="tp_ex")
                    nchunk = min(4, NCH - 4 * g)
                    for j in range(nchunk):
                        c = 4 * g + j
                        nc.tensor.transpose(tp[:, 128 * j:128 * (j + 1)],
                                            exT[:, 128 * c:128 * (c + 1)], idbf)
                    nc.scalar.activation(exE[:, 512 * g:512 * g + 128 * nchunk],
                                          tp[:, :128 * nchunk], AF.Copy)

                dn = ppd.tile([LA, NE * N_HEADS], F32, tag="dnp")
                for c in range(NCH):
                    nc.tensor.matmul(dn[:], cb("ohEc")[:, LA * c:LA * (c + 1)],
                                     exE[:, 128 * c:128 * (c + 1)],
                                     start=(c == 0), stop=(c == NCH - 1))
                dnS = pw.tile([LA, NE * N_HEADS], F32, tag="dnS")
                nc.vector.tensor_scalar_max(dnS[:], dn[:], 1e-30)
                nc.vector.reciprocal(rden[:], dnS[:])

                NG = 4  # n-groups of 8 for numer accumulation
                for r in range(NG):
                    npt = ppn.tile([LA, 1024], F32, tag="numerp")
                    for c in range(NCH):
                        exv = pw.tile([128, 1024], BF16, tag="exv")
                        # exE slice for n in [8r, 8r+8): free offset 4*8r = 32r
                        src_ex = exE[:, 128 * c + 32 * r:128 * c + 32 * r + 32]
                        src_ex = src_ex.rearrange("p (n h) -> p n h", h=N_HEADS)
                        src_ex = src_ex.unsqueeze(2).to_broadcast((128, 8, HEAD_DIM, N_HEADS))
                        src_v = vE[:, 128 * c:128 * (c + 1)]
                        src_v = src_v.rearrange("p (d h) -> p d h", h=N_HEADS)
                        src_v = src_v.unsqueeze(1).to_broadcast((128, 8, HEAD_DIM, N_HEADS))
                        dst = exv[:].rearrange("p (n d h) -> p n d h",
                                               n=8, d=HEAD_DIM, h=N_HEADS)
                        nc.vector.tensor_tensor(dst, src_ex, src_v, op=ALU.mult)
                        for half in range(2):
                            hs = slice(512 * half, 512 * (half + 1))
                            nc.tensor.matmul(npt[:, hs],
                                             cb("ohEc")[:, LA * c:LA * (c + 1)],
                                             exv[:, hs],
                                             start=(c == 0), stop=(c == NCH - 1))
                    # normalize into attnS
                    npt4 = npt[:].rearrange("p (n d h) -> p n d h",
                                            n=8, d=HEAD_DIM, h=N_HEADS)
                    rd4 = rden[:, 32 * r:32 * r + 32]
                    rd4 = rd4.rearrange("p (n h) -> p n h", h=N_HEADS)
                    rd4 = rd4.unsqueeze(2).to_broadcast((LA, 8, HEAD_DIM, N_HEADS))
                    at4 = attnS[:, 1024 * r:1024 * (r + 1)]
                    at4 = at4.rearrange("p (n d h) -> p n d h",
                                        n=8, d=HEAD_DIM, h=N_HEADS)
                    nc.vector.tensor_tensor(at4, npt4, rd4, op=ALU.mult)

            # =========== Phase E: out-proj ===========
            with tc.tile_pool(name="psO", bufs=2, space="PSUM") as ppo:
                for g in range(4):
                    tp = ppo.tile([128, 512], BF16, tag="tp_o")
                    for j in range(8):
                        n = 8 * g + j
                        nc.tensor.transpose(tp[:, 64 * j:64 * (j + 1)],
                                            attnS[:, 128 * n:128 * (n + 1)],
                                            idbf[0:LA, 0:LA])
                    nc.scalar.activation(oT[:, 512 * g:512 * (g + 1)], tp[:],
                                          AF.Copy)
                outT = pl.tile([128, OUT_COLS], F32, tag="outT")
                for g in range(4):
                    sl = slice(512 * g, 512 * (g + 1))
                    lo1 = ppo.tile([128, 512], F32, tag="o_l1")
                    nc.tensor.matmul(lo1[:], cb("w_o1"), oT[:, sl],
                                     start=True, stop=True)
                    xo = pw.tile([128, 512], BF16, tag="o_x")
                    nc.scalar.activation(xo[:], lo1[:], AF.Silu,
                                         bias=cf("b_o1")[:, 0:1])
                    lo2 = ppo.tile([128, 512], F32, tag="o_l2")
                    nc.tensor.matmul(lo2[:], cb("w_o2"), xo[:],
                                     start=True, stop=True)
                    nc.scalar.activation(outT[:, sl], lo2[:], AF.Identity,
                                         bias=cf("b_o2")[:, 0:1])
                    nc.sync.dma_start(out_d[:, sl], outT[:, sl])

    _split_waits(nc)
    return nc


# ---------------------------------------------------------------------------
# Host side
# ---------------------------------------------------------------------------

_NC_CACHE = None


def _get_nc():
    global _NC_CACHE
    if _NC_CACHE is None:
        _NC_CACHE = build_nc()
    return _NC_CACHE


def _np(x):
    return np.asarray(x)


def _prep_weights(params, e_feat):
    """Shared (non-per-core) pieces of the packed const arrays."""
    z = {}
    def L(mlp, i):
        W, b = mlp[i]
        return _np(W).astype(np.float32), _np(b).astype(np.float32)
    for name, mlp in (("k", params["key_mlp"]), ("v", params["value_mlp"])):
        W1, b1 = L(mlp, 0); W2, b2 = L(mlp, 1); W3, b3 = L(mlp, 2)
        z[f"w_{name}1a"] = W1[:128]
        z[f"w_{name}1b"] = W1[128:128 + KVIN]
        z[f"w_{name}1br"] = W1[128 + KVIN:]
        z[f"b_{name}1"] = b1[:, None]
        z[f"w_{name}2"] = W2
        z[f"b_{name}2"] = b2[:, None]
        z[f"w_{name}3"] = W3
        z[f"b_{name}3"] = b3[:, None]
    # d-outer permutation for v outputs / o1 rows: new row 4*d+h <- old 32*h+d
    l = np.arange(128)
    perm_pos = (l % 32) * 4 + (l // 32)
    inv = np.empty(128, np.int64)
    inv[perm_pos] = l
    W3, b3 = L(params["value_mlp"], 2)
    z["w_v3"] = W3[:, inv]
    z["b_v3"] = b3[inv][:, None]

    Wq1, bq1 = L(params["query_mlp"], 0)
    z["w_q1a"] = Wq1[:128]
    z["efT1"] = np.concatenate([_np(e_feat).astype(np.float32).T,
                                np.ones((1, NE), np.float32)], 0)
    z["w_q1b"] = np.concatenate([Wq1[128:], bq1[None, :]], 0)
    Wq2, bq2 = L(params["query_mlp"], 1)
    Wq3, bq3 = L(params["query_mlp"], 2)
    z["w_q2"] = Wq2
    z["b_q2"] = bq2[:, None]
    z["w_q3"] = Wq3
    bq3blk = np.zeros((128, N_HEADS), np.float32)
    for hh in range(N_HEADS):
        bq3blk[32 * hh:32 * hh + 32, hh] = bq3[32 * hh:32 * hh + 32]
    z["bq3blk"] = bq3blk
    C4 = np.zeros((N_HEADS, 128), np.float32)
    for hh in range(N_HEADS):
        C4[hh, hh::N_HEADS] = 1.0
    z["C4"] = C4

    Wo1, bo1 = L(params["out_proj"], 0)
    Wo2, bo2 = L(params["out_proj"], 1)
    z["w_o1"] = Wo1[inv, :]
    z["b_o1"] = bo1[:, None]
    z["w_o2"] = Wo2
    z["b_o2"] = bo2[:, None]

    centers = np.linspace(0.0, ATT_CUTOFF, RBF_DIM).astype(np.float32)
    z["negC"] = -centers[:, None]
    z["idbf"] = np.eye(128, dtype=np.float32)

    # B_n stationaries, chunked (128, 128*NE): B_n[(h,d), 4n+h] = 1
    Bnc = np.zeros((128, 128 * NE), np.float32)
    hh = np.arange(128) // 32
    for n in range(NE):
        Bnc[np.arange(128), 128 * n + 4 * n + hh] = 1.0
    z["Bnc"] = Bnc.astype(ml_dtypes.bfloat16)
    return z


def _host_prep(inputs):
    h = _np(inputs["h"]).astype(np.float32)
    zt = _np(inputs["z"])
    mask = _np(inputs["mask"]).astype(bool)
    e_feat = _np(inputs["e_feat"]).astype(np.float32)
    src = _np(inputs["att_src"]).astype(np.int64)
    dst = _np(inputs["att_dst"]).astype(np.int64)
    dist = _np(inputs["att_dist"]).astype(np.float32)
    params = inputs["params"]

    B, N, H = h.shape
    flat = B * N
    h_flat = h.reshape(flat, H)
    z_flat = zt.reshape(flat)
    mask_flat = mask.reshape(flat)
    edge_active = mask_flat[src] & mask_flat[dst]
    z_emb = _np(params["z_emb"]).astype(np.float32)

    wz = _prep_weights(params, e_feat)

    # shared packed f32 consts
    cf32 = np.zeros((128, CF32_W), np.float32)
    for name, p, w in CF32_ENTRIES:
        cf32[0:p, CF32_OFF[name]:CF32_OFF[name] + w] = wz[name]

    order = np.argsort(src, kind="stable")
    in_maps = []
    for c in range(NCORES):
        lo, hi = c * LA, (c + 1) * LA
        sel = order[(src[order] >= lo) & (src[order] < hi)]
        ne = sel.shape[0]
        if ne > EP:
            raise ValueError(f"core {c}: {ne} edges exceeds EP={EP}")
        e_src = np.zeros(EP, np.int64)
        e_dst = np.zeros(EP, np.int64)
        e_dist = np.full(EP, 2.5, np.float32)
        e_val = np.zeros(EP, bool)
        e_src[:ne] = src[sel] - lo
        e_dst[:ne] = dst[sel]
        e_dist[:ne] = dist[sel]
        e_val[:ne] = edge_active[sel]

        hdstT = np.ascontiguousarray(h_flat[e_dst].T)
        zr = z_emb[z_flat[e_dst]]
        is_self = ((e_dst == (e_src + lo)) & e_val).astype(np.float32)
        xkvbT = np.concatenate([zr.T, is_self[None, :]], 0).astype(np.float32)

        ohT = np.zeros((LA, EP), np.float32)
        vi = np.nonzero(e_val)[0]
        ohT[e_src[vi], vi] = 1.0
        ohEc = np.ascontiguousarray(
            ohT.reshape(LA, NCH, 128).transpose(2, 1, 0).reshape(128, NCH * LA))

        cbf = np.zeros((128, CBF_W), np.float32)
        core_arrays = dict(wz)
        core_arrays["h_locT"] = h_flat[lo:hi].T
        core_arrays["ohEc"] = ohEc
        for name, p, w in CBF_ENTRIES:
            cbf[0:p, CBF_OFF[name]:CBF_OFF[name] + w] = core_arrays[name]

        m = {
            "d16": np.broadcast_to(e_dist[None, :], (RBF_DIM, EP)).copy(),
            "dE": np.ascontiguousarray(e_dist.reshape(NCH, 128).T),
            "xkvbT": xkvbT.astype(ml_dtypes.bfloat16),
            "hdstT": hdstT.astype(ml_dtypes.bfloat16),
            "cf32": cf32,
            "cbf": cbf.astype(ml_dtypes.bfloat16),
            "ohT": ohT.astype(ml_dtypes.bfloat16),
            "Bnc": wz["Bnc"],
        }
        in_maps.append(m)
    return in_maps


def kernel(**inputs):
    from concourse import bass_utils
    nc = _get_nc()
    in_maps = _host_prep(inputs)
    res = bass_utils.run_bass_kernel_spmd(nc, in_maps, core_ids=list(range(NCORES)))
    outs = []
    for c in range(NCORES):
        outT = np.asarray(res.results[c]["out"])          # (128, 2048) cols=(n,a)
        outs.append(outT.reshape(128, NE, LA).transpose(2, 1, 0))
    full = np.concatenate(outs, 0)                        # (512, 32, 128)
    return np.ascontiguousarray(full.reshape(8, 64, NE, LATENT).astype(np.float32))


# revision 25
# speedup vs baseline: 1.2232x; 1.2232x over previous
# Trainium2 Bass kernel for nn_AllAtomAtomAttention (sparse edge attention).
#
# Sharding: atoms (512) are split contiguously over 8 cores (64 each); edges
# are routed to the core owning their source atom (segment softmax/sum stay
# device-local), padded to EP per core. MLP weights are replicated.
#
# Device algorithm (per core), all feature-major (features on partitions):
#   - rbf/cutoff from dist on ACT; kv-MLPs on PE (bf16), full-width silus
#   - q-MLP layer 1 factorized: hq = h_loc @ Wq1a expanded to edges via a
#     one-hot matmul; e_feat part enters as the per-partition ACT bias of the
#     layer-1 silu (n-major passes)
#   - scores accumulate in PSUM via per-n block-selector stationaries (B_n)
#     applied to the elementwise q*k product (DVE); softmax max-subtraction is
#     skipped (mathematically invariant; scores are O(10)); final-layer q bias
#     enters via a separate k-contraction (corr) matmul; SCORE_SCALE is
#     applied in fp32 via the Exp activation's scale immediate
#   - exp on ACT; segment sums (denominator and attn-weighted values) as
#     matmuls against per-edge one-hot matrices; normalization after the sum
#   - out-proj MLP on PE; host reassembles/transposes the output
#
# Masked/padded edges have all-zero one-hot columns, so they drop out of every
# segment sum exactly.

import numpy as np
import ml_dtypes

import concourse.bass as bass
import concourse.mybir as mybir
import concourse.tile as tile

F32 = mybir.dt.float32
BF16 = mybir.dt.bfloat16
AF = mybir.ActivationFunctionType
ALU = mybir.AluOpType

ATT_CUTOFF = 5.0
RBF_DIM = 16
N_HEADS = 4
LATENT = 128
HEAD_DIM = 32
ATOM_DIM = 128
E_DIM = 16
Z_EMB_DIM = 32
SCORE_SCALE = HEAD_DIM ** (-0.5)

NCORES = 8
LA = 64            # atoms per core
EP = 1152          # padded edges per core
NCH = EP // 128    # 128-edge chunks
NE = 32            # edge-feature contexts
SLICES = [(0, 512), (512, 512), (1024, 128)]  # EP split into <=512 runs
OUT_COLS = NE * LA  # 2048 tokens per core in out-proj
KVIN = Z_EMB_DIM + 1  # 33

# Packed bf16 constants: (name, partitions, cols)
CBF_ENTRIES = [
    ("w_k1a", 128, 128), ("w_k1b", KVIN, 128), ("w_k1br", RBF_DIM, 128),
    ("w_k2", 128, 128), ("w_k3", 128, 128),
    ("w_v1a", 128, 128), ("w_v1b", KVIN, 128), ("w_v1br", RBF_DIM, 128),
    ("w_v2", 128, 128), ("w_v3", 128, 128),
    ("w_q1a", 128, 128), ("w_q1b", E_DIM + 1, 128),
    ("w_q2", 128, 128), ("w_q3", 128, 128),
    ("w_o1", 128, 128), ("w_o2", 128, 128),
    ("efT1", E_DIM + 1, NE),
    ("h_locT", 128, LA),          # per-core
    ("ohEc", 128, LA * NCH),      # per-core
    ("idbf", 128, 128),
]
CBF_OFF = {}
_o = 0
for _n, _p, _w in CBF_ENTRIES:
    CBF_OFF[_n] = _o
    _o += _w
CBF_W = _o

# Packed f32 constants
CF32_ENTRIES = [
    ("b_k1", 128, 1), ("b_k2", 128, 1), ("b_k3", 128, 1),
    ("b_v1", 128, 1), ("b_v2", 128, 1), ("b_v3", 128, 1),
    ("b_q2", 128, 1), ("b_o1", 128, 1), ("b_o2", 128, 1),
    ("negC", RBF_DIM, 1), ("bq3blk", 128, N_HEADS), ("C4", N_HEADS, 128),
]
CF32_OFF = {}
_o = 0
for _n, _p, _w in CF32_ENTRIES:
    CF32_OFF[_n] = _o
    _o += _w
CF32_W = _o


def _edge_slices(width):
    o = 0
    res = []
    while o < EP:
        w = min(width, EP - o)
        res.append((o, w))
        o += w
    return res


# ---------------------------------------------------------------------------
# Device program
# ---------------------------------------------------------------------------

def _split_waits(nc, max_waits=1):
    """Walrus legalization: the TRN2 codegen rejects instructions carrying
    multiple sync waits (LDWEIGHTS/table-load companions consume the slot).
    Hoist extra waits onto same-engine Drain instructions placed just before."""
    n = 0
    for fn in nc.m.functions:
        for blk in fn.blocks:
            out = []
            for inst in blk.instructions:
                si = inst.sync_info
                if si is not None and si.on_wait and len(si.on_wait) > max_waits:
                    waits = list(si.on_wait)
                    for w in waits[:-max_waits]:
                        n += 1
                        out.append(mybir.InstDrain(
                            name=f"WSPLIT-{n}",
                            engine=inst.engine,
                            ins=[], outs=[],
                            sync_info=mybir.SyncInfo(on_wait=[w], on_update=[]),
                        ))
                    inst.sync_info = mybir.SyncInfo(
                        on_wait=waits[-max_waits:], on_update=list(si.on_update))
                out.append(inst)
            blk.instructions = out
    return n


def build_nc() -> bass.Bass:
    nc = bass.Bass()

    d = {}
    def din(name, shape, dt=F32):
        d[name] = nc.dram_tensor(name, shape, dt, kind="ExternalInput")
        return d[name]

    din("d16", (RBF_DIM, EP))
    din("dE", (128, NCH))
    din("xkvbT", (KVIN, EP), BF16)
    din("hdstT", (ATOM_DIM, EP), BF16)
    din("cf32", (128, CF32_W))
    din("cbf", (128, CBF_W), BF16)
    din("ohT", (LA, EP), BF16)
    din("Bnc", (128, 128 * NE), BF16)
    out_d = nc.dram_tensor("out", (128, OUT_COLS), F32, kind="ExternalOutput")

    with tile.TileContext(nc) as tc:
        with (
            tc.tile_pool(name="const", bufs=1) as pc,
            tc.tile_pool(name="long", bufs=1) as pl,
            tc.tile_pool(name="work", bufs=3) as pw,
        ):
            # ---- inputs into SBUF (ordered: phase-A needs first) ----
            d16 = pl.tile([RBF_DIM, EP], F32, tag="d16")
            nc.sync.dma_start(d16[:], d["d16"][:])
            dE = pl.tile([128, NCH], F32, tag="dE")
            nc.sync.dma_start(dE[:], d["dE"][:])
            xkvb = pl.tile([KVIN, EP], BF16, tag="xkvb")
            nc.sync.dma_start(xkvb[:], d["xkvbT"][:])
            hdstT = pl.tile([ATOM_DIM, EP], BF16, tag="hdstT")
            nc.sync.dma_start(hdstT[:], d["hdstT"][:])
            cf32 = pc.tile([128, CF32_W], F32, tag="cf32")
            nc.sync.dma_start(cf32[:], d["cf32"][:])
            cbf = pc.tile([128, CBF_W], BF16, tag="cbf")
            nc.sync.dma_start(cbf[:], d["cbf"][:])
            ohT_s = pl.tile([LA, EP], BF16, tag="ohT")
            nc.sync.dma_start(ohT_s[:], d["ohT"][:])
            Bnc = pc.tile([128, 128 * NE], BF16, tag="Bnc")
            nc.gpsimd.dma_start(Bnc[:], d["Bnc"][:])

            def cb(name):
                _, p, w = next(e for e in CBF_ENTRIES if e[0] == name)
                return cbf[0:p, CBF_OFF[name]:CBF_OFF[name] + w]

            def cf(name):
                _, p, w = next(e for e in CF32_ENTRIES if e[0] == name)
                return cf32[0:p, CF32_OFF[name]:CF32_OFF[name] + w]

            idbf = cb("idbf")

            # long-lived activations
            kT = pl.tile([128, EP], F32, tag="kT")
            vT = pl.tile([128, EP], BF16, tag="vT")
            vE = pl.tile([128, EP], BF16, tag="vE")
            exT = pl.tile([128, EP], BF16, tag="exT")
            exE = pl.tile([128, EP], BF16, tag="exE")
            efWT_S = pl.tile([128, NE], F32, tag="efWT_S")
            corrS = pl.tile([N_HEADS, EP], F32, tag="corrS")
            rden = pl.tile([LA, NE * N_HEADS], F32, tag="rden")
            attnS = pl.tile([LA, NE * LATENT], BF16, tag="attnS")
            oT = pl.tile([128, OUT_COLS], BF16, tag="oT")
            rbft = pl.tile([RBF_DIM, EP], BF16, tag="rbft")

            centers = np.linspace(0.0, ATT_CUTOFF, RBF_DIM)
            coeff = float(-0.5 / (centers[1] - centers[0]) ** 2)

            # =========== Phase A: rbf, cutoff, kv MLPs (full-width) ========
            with tc.tile_pool(name="psA", bufs=2, space="PSUM") as ppa:
                # rbf_j = exp(coeff*(d - c_j)^2)
                t16 = pw.tile([RBF_DIM, EP], F32, tag="t16")
                nc.scalar.activation(t16[:], d16[:], AF.Square,
                                     bias=cf("negC")[:, 0:1])
                nc.scalar.activation(rbft[:], t16[:], AF.Exp, scale=coeff)

                # cutoff = 0.5*sin(pi/c*d + pi/2) + 0.5, edge-major (128, NCH)
                sinargE = pw.tile([128, NCH], F32, tag="sinargE")
                nc.vector.tensor_scalar(sinargE[:], dE[:],
                                        float(np.pi / ATT_CUTOFF),
                                        float(np.pi / 2),
                                        op0=ALU.mult, op1=ALU.add)
                cutE = pw.tile([128, NCH], F32, tag="cutE")
                nc.scalar.activation(cutE[:], sinargE[:], AF.Sin)
                nc.vector.tensor_scalar(cutE[:], cutE[:], 0.5, 0.5,
                                        op0=ALU.mult, op1=ALU.add)

                # kv MLPs, full-width psum, k/v layers interleaved
                l1s, x1s, l2s, x2s, l3s = {}, {}, {}, {}, {}
                for p in ("k", "v"):
                    l1s[p] = ppa.tile([128, EP], F32, tag="pA", name=f"l1{p}")
                    for o, w in SLICES:
                        nc.tensor.matmul(l1s[p][:, o:o + w], cb(f"w_{p}1a"),
                                         hdstT[:, o:o + w], start=True, stop=False)
                        nc.tensor.matmul(l1s[p][:, o:o + w], cb(f"w_{p}1b"),
                                         xkvb[:, o:o + w], start=False, stop=False)
                        nc.tensor.matmul(l1s[p][:, o:o + w], cb(f"w_{p}1br"),
                                         rbft[:, o:o + w], start=False, stop=True)
                for p in ("k", "v"):
                    x1s[p] = pw.tile([128, EP], BF16, tag=f"kv_x1{p}", name=f"x1{p}")
                    nc.scalar.activation(x1s[p][:], l1s[p][:], AF.Silu,
                                         bias=cf(f"b_{p}1")[:, 0:1])
                for p in ("k", "v"):
                    l2s[p] = ppa.tile([128, EP], F32, tag="pA", name=f"l2{p}")
                    for o, w in SLICES:
                        nc.tensor.matmul(l2s[p][:, o:o + w], cb(f"w_{p}2"),
                                         x1s[p][:, o:o + w], start=True, stop=True)
                for p in ("k", "v"):
                    x2s[p] = pw.tile([128, EP], BF16, tag=f"kv_x2{p}", name=f"x2{p}")
                    nc.scalar.activation(x2s[p][:], l2s[p][:], AF.Silu,
                                         bias=cf(f"b_{p}2")[:, 0:1])
                for p in ("k", "v"):
                    l3s[p] = ppa.tile([128, EP], F32, tag="pA", name=f"l3{p}")
                    for o, w in SLICES:
                        nc.tensor.matmul(l3s[p][:, o:o + w], cb(f"w_{p}3"),
                                         x2s[p][:, o:o + w], start=True, stop=True)
                nc.scalar.activation(kT[:], l3s["k"][:], AF.Identity,
                                     bias=cf("b_k3")[:, 0:1])
                nc.scalar.activation(vT[:], l3s["v"][:], AF.Identity,
                                     bias=cf("b_v3")[:, 0:1])

                # transpose v -> edge-major vE, applying cutoff as the
                # per-partition (= per-edge) scale of the PSUM->SBUF copy
                for g in range((NCH + 3) // 4):
                    tp = ppa.tile([128, 512], BF16, tag="pT")
                    nchunk = min(4, NCH - 4 * g)
                    for j in range(nchunk):
                        c = 4 * g + j
                        nc.tensor.transpose(tp[:, 128 * j:128 * (j + 1)],
                                            vT[:, 128 * c:128 * (c + 1)], idbf)
                    for j in range(nchunk):
                        c = 4 * g + j
                        nc.scalar.activation(vE[:, 128 * c:128 * (c + 1)],
                                             tp[:, 128 * j:128 * (j + 1)], AF.Copy,
                                             scale=cutE[:, c:c + 1])
                cr = ppa.tile([N_HEADS, EP], F32, tag="pA")
                for o, w in SLICES:
                    nc.tensor.matmul(cr[:, o:o + w], cf("bq3blk"),
                                     kT[:, o:o + w], start=True, stop=True)
                nc.vector.tensor_copy(corrS[:], cr[:])

                # hq path: atom-level (feature-major, 64 atom columns)
                hqp = ppa.tile([128, LA], F32, tag="pT")
                nc.tensor.matmul(hqp[:], cb("w_q1a"), cb("h_locT"),
                                 start=True, stop=True)
                hqA_S = pw.tile([128, LA], F32, tag="hqA_S")
                nc.vector.tensor_copy(hqA_S[:], hqp[:])
                efp = ppa.tile([128, NE], F32, tag="pT")
                nc.tensor.matmul(efp[:], cb("w_q1b"), cb("efT1"),
                                 start=True, stop=True)
                nc.vector.tensor_copy(efWT_S[:], efp[:])

            # ====== Phase C1: atom-level q-MLP (2048 distinct (n,atom) rows) =====
            q3ATb = pl.tile([LA, 128 * NE], BF16, tag="q3ATb")
            with (
                tc.tile_pool(name="psC1", bufs=2, space="PSUM") as pc1,
                tc.tile_pool(name="psC1b", bufs=2, space="PSUM") as pc1b,
            ):
                x1all = pw.tile([128, LA * NE], BF16, tag="x1all")
                l1all = pw.tile([128, LA * NE], F32, tag="l1all")
                a_hq = hqA_S[:].unsqueeze(1).to_broadcast((128, NE, LA))
                a_ef = efWT_S[:].unsqueeze(2).to_broadcast((128, NE, LA))
                dst1 = l1all[:].rearrange("p (n a) -> p n a", n=NE)
                nc.vector.tensor_tensor(dst1, a_ef, a_hq, op=ALU.add)
                nc.scalar.activation(x1all[:], l1all[:], AF.Silu)
                x2all = pw.tile([128, LA * NE], BF16, tag="x2all")
                for q in range(4):
                    sl = slice(512 * q, 512 * (q + 1))
                    l2 = pc1.tile([128, 512], F32, tag="q_l2")
                    nc.tensor.matmul(l2[:], cb("w_q2"), x1all[:, sl],
                                     start=True, stop=True)
                    nc.scalar.activation(x2all[:, sl], l2[:], AF.Silu,
                                         bias=cf("b_q2")[:, 0:1])
                for r in range(4):
                    q3p = pc1b.tile([LA, 1024], F32, tag="q3p")
                    for j in range(8):
                        n = 8 * r + j
                        nc.tensor.matmul(q3p[:, 128 * j:128 * (j + 1)],
                                         x2all[:, LA * n:LA * (n + 1)],
                                         cb("w_q3"), start=True, stop=True)
                    nc.scalar.activation(q3ATb[:, 1024 * r:1024 * (r + 1)],
                                         q3p[:], AF.Copy)

            # ====== Phase C2: expand to edges, scores ======
            with (
                tc.tile_pool(name="psS", bufs=1, space="PSUM") as pps,
                tc.tile_pool(name="psE", bufs=2, space="PSUM") as pse,
            ):
                scores = pps.tile([128, EP], F32, tag="scores")
                for o, w in SLICES:
                    nc.tensor.matmul(scores[:, o:o + w], cf("C4"),
                                     corrS[:, o:o + w], start=True, stop=False)
                HW2 = EP // 2  # 576
                BSL = {0: [(0, 512), (512, 64)], 1: [(576, 448), (1024, 128)]}
                for n in range(NE):
                    qk = pw.tile([128, EP], BF16, tag="q_qk")
                    for hf in (0, 1):
                        base = HW2 * hf
                        q3E = pse.tile([128, HW2], F32, tag="q3E")
                        nc.tensor.matmul(q3E[:, 0:512],
                                         q3ATb[:, 128 * n:128 * (n + 1)],
                                         ohT_s[:, base:base + 512],
                                         start=True, stop=True)
                        nc.tensor.matmul(q3E[:, 512:HW2],
                                         q3ATb[:, 128 * n:128 * (n + 1)],
                                         ohT_s[:, base + 512:base + HW2],
                                         start=True, stop=True)
                        # (HW2 = 576: slices 512 + 64)
                        nc.vector.tensor_tensor(qk[:, base:base + HW2], q3E[:],
                                                kT[:, base:base + HW2],
                                                op=ALU.mult)
                        for o, w in BSL[hf]:
                            nc.tensor.matmul(scores[:, o:o + w],
                                             Bnc[:, 128 * n:128 * (n + 1)],
                                             qk[:, o:o + w], start=False,
                                             stop=(n == NE - 1 and o + w == EP))
                # exp (SCORE_SCALE applied here in fp32)
                nc.scalar.activation(exT[:], scores[:], AF.Exp, scale=SCORE_SCALE)

            # =========== Phase D: transposes, segment sums ===========
            with (
                tc.tile_pool(name="psT", bufs=2, space="PSUM") as ppt,
                tc.tile_pool(name="psD", bufs=1, space="PSUM") as ppd,
                tc.tile_pool(name="psN", bufs=2, space="PSUM") as ppn,
            ):
                for g in range((NCH + 3) // 4):
                    tp = ppt.tile([128, 512], BF16, tag="tp_ex")
                    nchunk = min(4, NCH - 4 * g)
                    for j in range(nchunk):
                        c = 4 * g + j
                        nc.tensor.transpose(tp[:, 128 * j:128 * (j + 1)],
                                            exT[:, 128 * c:128 * (c + 1)], idbf)
                    nc.scalar.activation(exE[:, 512 * g:512 * g + 128 * nchunk],
                                          tp[:, :128 * nchunk], AF.Copy)

                dn = ppd.tile([LA, NE * N_HEADS], F32, tag="dnp")
                for c in range(NCH):
                    nc.tensor.matmul(dn[:], cb("ohEc")[:, LA * c:LA * (c + 1)],
                                     exE[:, 128 * c:128 * (c + 1)],
                                     start=(c == 0), stop=(c == NCH - 1))
                dnS = pw.tile([LA, NE * N_HEADS], F32, tag="dnS")
                nc.vector.tensor_scalar_max(dnS[:], dn[:], 1e-30)
                nc.vector.reciprocal(rden[:], dnS[:])

                NG = 4  # n-groups of 8 for numer accumulation
                for r in range(NG):
                    npt = ppn.tile([LA, 1024], F32, tag="numerp")
                    for c in range(NCH):
                        exv = pw.tile([128, 1024], BF16, tag="exv")
                        # exE slice for n in [8r, 8r+8): free offset 4*8r = 32r
                        src_ex = exE[:, 128 * c + 32 * r:128 * c + 32 * r + 32]
                        src_ex = src_ex.rearrange("p (n h) -> p n h", h=N_HEADS)
                        src_ex = src_ex.unsqueeze(2).to_broadcast((128, 8, HEAD_DIM, N_HEADS))
                        src_v = vE[:, 128 * c:128 * (c + 1)]
                        src_v = src_v.rearrange("p (d h) -> p d h", h=N_HEADS)
                        src_v = src_v.unsqueeze(1).to_broadcast((128, 8, HEAD_DIM, N_HEADS))
                        dst = exv[:].rearrange("p (n d h) -> p n d h",
                                               n=8, d=HEAD_DIM, h=N_HEADS)
                        nc.vector.tensor_tensor(dst, src_ex, src_v, op=ALU.mult)
                        for half in range(2):
                            hs = slice(512 * half, 512 * (half + 1))
                            nc.tensor.matmul(npt[:, hs],
                                             cb("ohEc")[:, LA * c:LA * (c + 1)],
                                             exv[:, hs],
                                             start=(c == 0), stop=(c == NCH - 1))
                    # normalize into attnS
                    npt4 = npt[:].rearrange("p (n d h) -> p n d h",
                                            n=8, d=HEAD_DIM, h=N_HEADS)
                    rd4 = rden[:, 32 * r:32 * r + 32]
                    rd4 = rd4.rearrange("p (n h) -> p n h", h=N_HEADS)
                    rd4 = rd4.unsqueeze(2).to_broadcast((LA, 8, HEAD_DIM, N_HEADS))
                    at4 = attnS[:, 1024 * r:1024 * (r + 1)]
                    at4 = at4.rearrange("p (n d h) -> p n d h",
                                        n=8, d=HEAD_DIM, h=N_HEADS)
                    nc.vector.tensor_tensor(at4, npt4, rd4, op=ALU.mult)

            # =========== Phase E: out-proj ===========
            with tc.tile_pool(name="psO", bufs=2, space="PSUM") as ppo:
                for g in range(4):
                    tp = ppo.tile([128, 512], BF16, tag="tp_o")
                    for j in range(8):
                        n = 8 * g + j
                        nc.tensor.transpose(tp[:, 64 * j:64 * (j + 1)],
                                            attnS[:, 128 * n:128 * (n + 1)],
                                            idbf[0:LA, 0:LA])
                    nc.scalar.activation(oT[:, 512 * g:512 * (g + 1)], tp[:],
                                          AF.Copy)
                outT = pl.tile([128, OUT_COLS], F32, tag="outT")
                for g in range(4):
                    sl = slice(512 * g, 512 * (g + 1))
                    lo1 = ppo.tile([128, 512], F32, tag="o_l1")
                    nc.tensor.matmul(lo1[:], cb("w_o1"), oT[:, sl],
                                     start=True, stop=True)
                    xo = pw.tile([128, 512], BF16, tag="o_x")
                    nc.scalar.activation(xo[:], lo1[:], AF.Silu,
                                         bias=cf("b_o1")[:, 0:1])
                    lo2 = ppo.tile([128, 512], F32, tag="o_l2")
                    nc.tensor.matmul(lo2[:], cb("w_o2"), xo[:],
                                     start=True, stop=True)
                    nc.scalar.activation(outT[:, sl], lo2[:], AF.Identity,
                                         bias=cf("b_o2")[:, 0:1])
                    nc.sync.dma_start(out_d[:, sl], outT[:, sl])

    _split_waits(nc)
    return nc


# ---------------------------------------------------------------------------
# Host side
# ---------------------------------------------------------------------------

_NC_CACHE = None


def _get_nc():
    global _NC_CACHE
    if _NC_CACHE is None:
        _NC_CACHE = build_nc()
    return _NC_CACHE


def _np(x):
    return np.asarray(x)


def _prep_weights(params, e_feat):
    """Shared (non-per-core) pieces of the packed const arrays."""
    z = {}
    def L(mlp, i):
        W, b = mlp[i]
        return _np(W).astype(np.float32), _np(b).astype(np.float32)
    for name, mlp in (("k", params["key_mlp"]), ("v", params["value_mlp"])):
        W1, b1 = L(mlp, 0); W2, b2 = L(mlp, 1); W3, b3 = L(mlp, 2)
        z[f"w_{name}1a"] = W1[:128]
        z[f"w_{name}1b"] = W1[128:128 + KVIN]
        z[f"w_{name}1br"] = W1[128 + KVIN:]
        z[f"b_{name}1"] = b1[:, None]
        z[f"w_{name}2"] = W2
        z[f"b_{name}2"] = b2[:, None]
        z[f"w_{name}3"] = W3
        z[f"b_{name}3"] = b3[:, None]
    # d-outer permutation for v outputs / o1 rows: new row 4*d+h <- old 32*h+d
    l = np.arange(128)
    perm_pos = (l % 32) * 4 + (l // 32)
    inv = np.empty(128, np.int64)
    inv[perm_pos] = l
    W3, b3 = L(params["value_mlp"], 2)
    z["w_v3"] = W3[:, inv]
    z["b_v3"] = b3[inv][:, None]

    Wq1, bq1 = L(params["query_mlp"], 0)
    z["w_q1a"] = Wq1[:128]
    z["efT1"] = np.concatenate([_np(e_feat).astype(np.float32).T,
                                np.ones((1, NE), np.float32)], 0)
    z["w_q1b"] = np.concatenate([Wq1[128:], bq1[None, :]], 0)
    Wq2, bq2 = L(params["query_mlp"], 1)
    Wq3, bq3 = L(params["query_mlp"], 2)
    z["w_q2"] = Wq2
    z["b_q2"] = bq2[:, None]
    z["w_q3"] = Wq3
    bq3blk = np.zeros((128, N_HEADS), np.float32)
    for hh in range(N_HEADS):
        bq3blk[32 * hh:32 * hh + 32, hh] = bq3[32 * hh:32 * hh + 32]
    z["bq3blk"] = bq3blk
    C4 = np.zeros((N_HEADS, 128), np.float32)
    for hh in range(N_HEADS):
        C4[hh, hh::N_HEADS] = 1.0
    z["C4"] = C4

    Wo1, bo1 = L(params["out_proj"], 0)
    Wo2, bo2 = L(params["out_proj"], 1)
    z["w_o1"] = Wo1[inv, :]
    z["b_o1"] = bo1[:, None]
    z["w_o2"] = Wo2
    z["b_o2"] = bo2[:, None]

    centers = np.linspace(0.0, ATT_CUTOFF, RBF_DIM).astype(np.float32)
    z["negC"] = -centers[:, None]
    z["idbf"] = np.eye(128, dtype=np.float32)

    # B_n stationaries, chunked (128, 128*NE): B_n[(h,d), 4n+h] = 1
    Bnc = np.zeros((128, 128 * NE), np.float32)
    hh = np.arange(128) // 32
    for n in range(NE):
        Bnc[np.arange(128), 128 * n + 4 * n + hh] = 1.0
    z["Bnc"] = Bnc.astype(ml_dtypes.bfloat16)
    return z


def _host_prep(inputs):
    h = _np(inputs["h"]).astype(np.float32)
    zt = _np(inputs["z"])
    mask = _np(inputs["mask"]).astype(bool)
    e_feat = _np(inputs["e_feat"]).astype(np.float32)
    src = _np(inputs["att_src"]).astype(np.int64)
    dst = _np(inputs["att_dst"]).astype(np.int64)
    dist = _np(inputs["att_dist"]).astype(np.float32)
    params = inputs["params"]

    B, N, H = h.shape
    flat = B * N
    h_flat = h.reshape(flat, H)
    z_flat = zt.reshape(flat)
    mask_flat = mask.reshape(flat)
    edge_active = mask_flat[src] & mask_flat[dst]
    z_emb = _np(params["z_emb"]).astype(np.float32)

    wz = _prep_weights(params, e_feat)

    # shared packed f32 consts
    cf32 = np.zeros((128, CF32_W), np.float32)
    for name, p, w in CF32_ENTRIES:
        cf32[0:p, CF32_OFF[name]:CF32_OFF[name] + w] = wz[name]

    order = np.argsort(src, kind="stable")
    in_maps = []
    for c in range(NCORES):
        lo, hi = c * LA, (c + 1) * LA
        sel = order[(src[order] >= lo) & (src[order] < hi)]
        ne = sel.shape[0]
        if ne > EP:
            raise ValueError(f"core {c}: {ne} edges exceeds EP={EP}")
        e_src = np.zeros(EP, np.int64)
        e_dst = np.zeros(EP, np.int64)
        e_dist = np.full(EP, 2.5, np.float32)
        e_val = np.zeros(EP, bool)
        e_src[:ne] = src[sel] - lo
        e_dst[:ne] = dst[sel]
        e_dist[:ne] = dist[sel]
        e_val[:ne] = edge_active[sel]

        hdstT = np.ascontiguousarray(h_flat[e_dst].T)
        zr = z_emb[z_flat[e_dst]]
        is_self = ((e_dst == (e_src + lo)) & e_val).astype(np.float32)
        xkvbT = np.concatenate([zr.T, is_self[None, :]], 0).astype(np.float32)

        ohT = np.zeros((LA, EP), np.float32)
        vi = np.nonzero(e_val)[0]
        ohT[e_src[vi], vi] = 1.0
        ohEc = np.ascontiguousarray(
            ohT.reshape(LA, NCH, 128).transpose(2, 1, 0).reshape(128, NCH * LA))

        cbf = np.zeros((128, CBF_W), np.float32)
        core_arrays = dict(wz)
        core_arrays["h_locT"] = h_flat[lo:hi].T
        core_arrays["ohEc"] = ohEc
        for name, p, w in CBF_ENTRIES:
            cbf[0:p, CBF_OFF[name]:CBF_OFF[name] + w] = core_arrays[name]

        m = {
            "d16": np.broadcast_to(e_dist[None, :], (RBF_DIM, EP)).copy(),
            "dE": np.ascontiguousarray(e_dist.reshape(NCH, 128).T),
            "xkvbT": xkvbT.astype(ml_dtypes.bfloat16),
            "hdstT": hdstT.astype(ml_dtypes.bfloat16),
            "cf32": cf32,
            "cbf": cbf.astype(ml_dtypes.bfloat16),
            "ohT": ohT.astype(ml_dtypes.bfloat16),
            "Bnc": wz["Bnc"],
        }
        in_maps.append(m)
    return in_maps


def kernel(**inputs):
    from concourse import bass_utils
    nc = _get_nc()
    in_maps = _host_prep(inputs)
    res = bass_utils.run_bass_kernel_spmd(nc, in_maps, core_ids=list(range(NCORES)))
    outs = []
    for c in range(NCORES):
        outT = np.asarray(res.results[c]["out"])          # (128, 2048) cols=(n,a)
        outs.append(outT.reshape(128, NE, LA).transpose(2, 1, 0))
    full = np.concatenate(outs, 0)                        # (512, 32, 128)
    return np.ascontiguousarray(full.reshape(8, 64, NE, LATENT).astype(np.float32))


# revision 27
# speedup vs baseline: 1.2797x; 1.0461x over previous
# Trainium2 Bass kernel for nn_AllAtomAtomAttention (sparse edge attention).
#
# Sharding: atoms (512) are split contiguously over 8 cores (64 each); edges
# are routed to the core owning their source atom (segment softmax/sum stay
# device-local), padded to EP per core. MLP weights are replicated.
#
# Device algorithm (per core), all feature-major (features on partitions):
#   - rbf/cutoff from dist on ACT; kv-MLPs on PE (bf16), full-width silus
#   - q-MLP layer 1 factorized: hq = h_loc @ Wq1a expanded to edges via a
#     one-hot matmul; e_feat part enters as the per-partition ACT bias of the
#     layer-1 silu (n-major passes)
#   - scores accumulate in PSUM via per-n block-selector stationaries (B_n)
#     applied to the elementwise q*k product (DVE); softmax max-subtraction is
#     skipped (mathematically invariant; scores are O(10)); final-layer q bias
#     enters via a separate k-contraction (corr) matmul; SCORE_SCALE is
#     applied in fp32 via the Exp activation's scale immediate
#   - exp on ACT; segment sums (denominator and attn-weighted values) as
#     matmuls against per-edge one-hot matrices; normalization after the sum
#   - out-proj MLP on PE; host reassembles/transposes the output
#
# Masked/padded edges have all-zero one-hot columns, so they drop out of every
# segment sum exactly.

import numpy as np
import ml_dtypes

import concourse.bass as bass
import concourse.mybir as mybir
import concourse.tile as tile

F32 = mybir.dt.float32
BF16 = mybir.dt.bfloat16
AF = mybir.ActivationFunctionType
ALU = mybir.AluOpType

ATT_CUTOFF = 5.0
RBF_DIM = 16
N_HEADS = 4
LATENT = 128
HEAD_DIM = 32
ATOM_DIM = 128
E_DIM = 16
Z_EMB_DIM = 32
SCORE_SCALE = HEAD_DIM ** (-0.5)

NCORES = 8
LA = 64            # atoms per core
EP = 1152          # padded edges per core
NCH = EP // 128    # 128-edge chunks
NE = 32            # edge-feature contexts
SLICES = [(0, 512), (512, 512), (1024, 128)]  # EP split into <=512 runs
OUT_COLS = NE * LA  # 2048 tokens per core in out-proj
KVIN = Z_EMB_DIM + 1  # 33

# Packed bf16 constants: (name, partitions, cols)
CBF_ENTRIES = [
    ("w_k1a", 128, 128), ("w_k1b", KVIN, 128), ("w_k1br", RBF_DIM, 128),
    ("w_k2", 128, 128), ("w_k3", 128, 128),
    ("w_v1a", 128, 128), ("w_v1b", KVIN, 128), ("w_v1br", RBF_DIM, 128),
    ("w_v2", 128, 128), ("w_v3", 128, 128),
    ("w_q1a", 128, 128), ("w_q1b", E_DIM + 1, 128),
    ("w_q2", 128, 128), ("w_q3", 128, 128),
    ("w_o1", 128, 128), ("w_o2", 128, 128),
    ("efT1", E_DIM + 1, NE),
    ("h_locT", 128, LA),          # per-core
    ("ohEc", 128, LA * NCH),      # per-core
    ("idbf", 128, 128),
]
CBF_OFF = {}
_o = 0
for _n, _p, _w in CBF_ENTRIES:
    CBF_OFF[_n] = _o
    _o += _w
CBF_W = _o

# Packed f32 constants
CF32_ENTRIES = [
    ("b_k1", 128, 1), ("b_k2", 128, 1), ("b_k3", 128, 1),
    ("b_v1", 128, 1), ("b_v2", 128, 1), ("b_v3", 128, 1),
    ("b_q2", 128, 1), ("b_o1", 128, 1), ("b_o2", 128, 1),
    ("negC", RBF_DIM, 1), ("bq3blk", 128, N_HEADS), ("C4", N_HEADS, 128),
]
CF32_OFF = {}
_o = 0
for _n, _p, _w in CF32_ENTRIES:
    CF32_OFF[_n] = _o
    _o += _w
CF32_W = _o


def _edge_slices(width):
    o = 0
    res = []
    while o < EP:
        w = min(width, EP - o)
        res.append((o, w))
        o += w
    return res


# ---------------------------------------------------------------------------
# Device program
# ---------------------------------------------------------------------------

def _split_waits(nc, max_waits=1):
    """Walrus legalization: the TRN2 codegen rejects instructions carrying
    multiple sync waits (LDWEIGHTS/table-load companions consume the slot).
    Hoist extra waits onto same-engine Drain instructions placed just before."""
    n = 0
    for fn in nc.m.functions:
        for blk in fn.blocks:
            out = []
            for inst in blk.instructions:
                si = inst.sync_info
                if si is not None and si.on_wait and len(si.on_wait) > max_waits:
                    waits = list(si.on_wait)
                    for w in waits[:-max_waits]:
                        n += 1
                        out.append(mybir.InstDrain(
                            name=f"WSPLIT-{n}",
                            engine=inst.engine,
                            ins=[], outs=[],
                            sync_info=mybir.SyncInfo(on_wait=[w], on_update=[]),
                        ))
                    inst.sync_info = mybir.SyncInfo(
                        on_wait=waits[-max_waits:], on_update=list(si.on_update))
                out.append(inst)
            blk.instructions = out
    return n


def build_nc() -> bass.Bass:
    nc = bass.Bass()

    d = {}
    def din(name, shape, dt=F32):
        d[name] = nc.dram_tensor(name, shape, dt, kind="ExternalInput")
        return d[name]

    din("d16", (RBF_DIM, EP))
    din("dE", (128, NCH))
    din("xkvbT", (KVIN, EP), BF16)
    din("hdstT", (ATOM_DIM, EP), BF16)
    din("cf32", (128, CF32_W))
    din("cbf", (128, CBF_W), BF16)
    din("ohT", (LA, EP), BF16)
    din("Bnc", (128, 128 * NE), BF16)
    out_d = nc.dram_tensor("out", (128, OUT_COLS), F32, kind="ExternalOutput")

    with tile.TileContext(nc) as tc:
        with (
            tc.tile_pool(name="const", bufs=1) as pc,
            tc.tile_pool(name="long", bufs=1) as pl,
            tc.tile_pool(name="work", bufs=3) as pw,
        ):
            # ---- inputs into SBUF (ordered: phase-A needs first) ----
            d16 = pl.tile([RBF_DIM, EP], F32, tag="d16")
            nc.sync.dma_start(d16[:], d["d16"][:])
            dE = pl.tile([128, NCH], F32, tag="dE")
            nc.sync.dma_start(dE[:], d["dE"][:])
            xkvb = pl.tile([KVIN, EP], BF16, tag="xkvb")
            nc.sync.dma_start(xkvb[:], d["xkvbT"][:])
            hdstT = pl.tile([ATOM_DIM, EP], BF16, tag="hdstT")
            nc.scalar.dma_start(hdstT[:], d["hdstT"][:])
            cf32 = pc.tile([128, CF32_W], F32, tag="cf32")
            nc.sync.dma_start(cf32[:], d["cf32"][:])
            cbf = pc.tile([128, CBF_W], BF16, tag="cbf")
            nc.sync.dma_start(cbf[:], d["cbf"][:])
            ohT_s = pl.tile([LA, EP], BF16, tag="ohT")
            nc.gpsimd.dma_start(ohT_s[:], d["ohT"][:])
            Bnc = pc.tile([128, 128 * NE], BF16, tag="Bnc")
            nc.gpsimd.dma_start(Bnc[:], d["Bnc"][:])

            def cb(name):
                _, p, w = next(e for e in CBF_ENTRIES if e[0] == name)
                return cbf[0:p, CBF_OFF[name]:CBF_OFF[name] + w]

            def cf(name):
                _, p, w = next(e for e in CF32_ENTRIES if e[0] == name)
                return cf32[0:p, CF32_OFF[name]:CF32_OFF[name] + w]

            idbf = cb("idbf")

            # long-lived activations
            kT = pl.tile([128, EP], F32, tag="kT")
            vT = pl.tile([128, EP], BF16, tag="vT")
            vE = pl.tile([128, EP], BF16, tag="vE")
            exT = pl.tile([128, EP], BF16, tag="exT")
            exE = pl.tile([128, EP], BF16, tag="exE")
            efWT_S = pl.tile([128, NE], F32, tag="efWT_S")
            corrS = pl.tile([N_HEADS, EP], F32, tag="corrS")
            rden = pl.tile([LA, NE * N_HEADS], F32, tag="rden")
            attnS = pl.tile([LA, NE * LATENT], BF16, tag="attnS")
            oT = pl.tile([128, OUT_COLS], BF16, tag="oT")
            rbft = pl.tile([RBF_DIM, EP], BF16, tag="rbft")

            centers = np.linspace(0.0, ATT_CUTOFF, RBF_DIM)
            coeff = float(-0.5 / (centers[1] - centers[0]) ** 2)

            # =========== Phase A: rbf, cutoff, kv MLPs (full-width) ========
            with tc.tile_pool(name="psA", bufs=2, space="PSUM") as ppa:
                # rbf_j = exp(coeff*(d - c_j)^2)
                t16 = pw.tile([RBF_DIM, EP], F32, tag="t16")
                nc.scalar.activation(t16[:], d16[:], AF.Square,
                                     bias=cf("negC")[:, 0:1])
                nc.scalar.activation(rbft[:], t16[:], AF.Exp, scale=coeff)

                # cutoff = 0.5*sin(pi/c*d + pi/2) + 0.5, edge-major (128, NCH)
                sinargE = pw.tile([128, NCH], F32, tag="sinargE")
                nc.vector.tensor_scalar(sinargE[:], dE[:],
                                        float(np.pi / ATT_CUTOFF),
                                        float(np.pi / 2),
                                        op0=ALU.mult, op1=ALU.add)
                cutE = pw.tile([128, NCH], F32, tag="cutE")
                nc.scalar.activation(cutE[:], sinargE[:], AF.Sin)
                nc.vector.tensor_scalar(cutE[:], cutE[:], 0.5, 0.5,
                                        op0=ALU.mult, op1=ALU.add)

                # hq path: atom-level (feature-major, 64 atom columns)
                hqp = ppa.tile([128, LA], F32, tag="pT")
                nc.tensor.matmul(hqp[:], cb("w_q1a"), cb("h_locT"),
                                 start=True, stop=True)
                hqA_S = pw.tile([128, LA], F32, tag="hqA_S")
                nc.vector.tensor_copy(hqA_S[:], hqp[:])
                efp = ppa.tile([128, NE], F32, tag="pT")
                nc.tensor.matmul(efp[:], cb("w_q1b"), cb("efT1"),
                                 start=True, stop=True)
                nc.vector.tensor_copy(efWT_S[:], efp[:])

                # kv MLPs, full-width psum, k/v layers interleaved
                l1s, x1s, l2s, x2s, l3s = {}, {}, {}, {}, {}
                for p in ("k", "v"):
                    l1s[p] = ppa.tile([128, EP], F32, tag="pA", name=f"l1{p}")
                    for o, w in SLICES:
                        nc.tensor.matmul(l1s[p][:, o:o + w], cb(f"w_{p}1a"),
                                         hdstT[:, o:o + w], start=True, stop=False)
                        nc.tensor.matmul(l1s[p][:, o:o + w], cb(f"w_{p}1b"),
                                         xkvb[:, o:o + w], start=False, stop=False)
                        nc.tensor.matmul(l1s[p][:, o:o + w], cb(f"w_{p}1br"),
                                         rbft[:, o:o + w], start=False, stop=True)
                for p in ("k", "v"):
                    x1s[p] = pw.tile([128, EP], BF16, tag=f"kv_x1{p}", name=f"x1{p}")
                    nc.scalar.activation(x1s[p][:], l1s[p][:], AF.Silu,
                                         bias=cf(f"b_{p}1")[:, 0:1])
                for p in ("k", "v"):
                    l2s[p] = ppa.tile([128, EP], F32, tag="pA", name=f"l2{p}")
                    for o, w in SLICES:
                        nc.tensor.matmul(l2s[p][:, o:o + w], cb(f"w_{p}2"),
                                         x1s[p][:, o:o + w], start=True, stop=True)
                for p in ("k", "v"):
                    x2s[p] = pw.tile([128, EP], BF16, tag=f"kv_x2{p}", name=f"x2{p}")
                    nc.scalar.activation(x2s[p][:], l2s[p][:], AF.Silu,
                                         bias=cf(f"b_{p}2")[:, 0:1])
                for p in ("k", "v"):
                    l3s[p] = ppa.tile([128, EP], F32, tag="pA", name=f"l3{p}")
                    for o, w in SLICES:
                        nc.tensor.matmul(l3s[p][:, o:o + w], cb(f"w_{p}3"),
                                         x2s[p][:, o:o + w], start=True, stop=True)
                nc.scalar.activation(kT[:], l3s["k"][:], AF.Identity,
                                     bias=cf("b_k3")[:, 0:1])
                nc.scalar.activation(vT[:], l3s["v"][:], AF.Identity,
                                     bias=cf("b_v3")[:, 0:1])

                # transpose v -> edge-major vE, applying cutoff as the
                # per-partition (= per-edge) scale of the PSUM->SBUF copy
                for g in range((NCH + 3) // 4):
                    tp = ppa.tile([128, 512], BF16, tag="pT")
                    nchunk = min(4, NCH - 4 * g)
                    for j in range(nchunk):
                        c = 4 * g + j
                        nc.tensor.transpose(tp[:, 128 * j:128 * (j + 1)],
                                            vT[:, 128 * c:128 * (c + 1)], idbf)
                    for j in range(nchunk):
                        c = 4 * g + j
                        nc.scalar.activation(vE[:, 128 * c:128 * (c + 1)],
                                             tp[:, 128 * j:128 * (j + 1)], AF.Copy,
                                             scale=cutE[:, c:c + 1])
                cr = ppa.tile([N_HEADS, EP], F32, tag="pA")
                for o, w in SLICES:
                    nc.tensor.matmul(cr[:, o:o + w], cf("bq3blk"),
                                     kT[:, o:o + w], start=True, stop=True)
                nc.vector.tensor_copy(corrS[:], cr[:])


            # ====== Phase C1: atom-level q-MLP (2048 distinct (n,atom) rows) =====
            q3ATb = pl.tile([LA, 128 * NE], BF16, tag="q3ATb")
            with (
                tc.tile_pool(name="psC1", bufs=2, space="PSUM") as pc1,
                tc.tile_pool(name="psC1b", bufs=2, space="PSUM") as pc1b,
            ):
                x1all = pw.tile([128, LA * NE], BF16, tag="x1all")
                l1all = pw.tile([128, LA * NE], F32, tag="l1all")
                a_hq = hqA_S[:].unsqueeze(1).to_broadcast((128, NE, LA))
                a_ef = efWT_S[:].unsqueeze(2).to_broadcast((128, NE, LA))
                dst1 = l1all[:].rearrange("p (n a) -> p n a", n=NE)
                nc.vector.tensor_tensor(dst1, a_ef, a_hq, op=ALU.add)
                nc.scalar.activation(x1all[:], l1all[:], AF.Silu)
                x2all = pw.tile([128, LA * NE], BF16, tag="x2all")
                for q in range(4):
                    sl = slice(512 * q, 512 * (q + 1))
                    l2 = pc1.tile([128, 512], F32, tag="q_l2")
                    nc.tensor.matmul(l2[:], cb("w_q2"), x1all[:, sl],
                                     start=True, stop=True)
                    nc.scalar.activation(x2all[:, sl], l2[:], AF.Silu,
                                         bias=cf("b_q2")[:, 0:1])
                for r in range(4):
                    q3p = pc1b.tile([LA, 1024], F32, tag="q3p")
                    for j in range(8):
                        n = 8 * r + j
                        nc.tensor.matmul(q3p[:, 128 * j:128 * (j + 1)],
                                         x2all[:, LA * n:LA * (n + 1)],
                                         cb("w_q3"), start=True, stop=True)
                    nc.scalar.activation(q3ATb[:, 1024 * r:1024 * (r + 1)],
                                         q3p[:], AF.Copy)

            # ====== Phase C2: expand to edges, scores ======
            with (
                tc.tile_pool(name="psS", bufs=1, space="PSUM") as pps,
                tc.tile_pool(name="psE", bufs=2, space="PSUM") as pse,
            ):
                scores = pps.tile([128, EP], F32, tag="scores")
                for o, w in SLICES:
                    nc.tensor.matmul(scores[:, o:o + w], cf("C4"),
                                     corrS[:, o:o + w], start=True, stop=False)
                HW2 = EP // 2  # 576
                BSL = {0: [(0, 512), (512, 64)], 1: [(576, 448), (1024, 128)]}
                for n in range(NE):
                    qk = pw.tile([128, EP], BF16, tag="q_qk")
                    for hf in (0, 1):
                        base = HW2 * hf
                        q3E = pse.tile([128, HW2], F32, tag="q3E")
                        nc.tensor.matmul(q3E[:, 0:512],
                                         q3ATb[:, 128 * n:128 * (n + 1)],
                                         ohT_s[:, base:base + 512],
                                         start=True, stop=True)
                        nc.tensor.matmul(q3E[:, 512:HW2],
                                         q3ATb[:, 128 * n:128 * (n + 1)],
                                         ohT_s[:, base + 512:base + HW2],
                                         start=True, stop=True)
                        # (HW2 = 576: slices 512 + 64)
                        nc.vector.tensor_tensor(qk[:, base:base + HW2], q3E[:],
                                                kT[:, base:base + HW2],
                                                op=ALU.mult)
                        for o, w in BSL[hf]:
                            nc.tensor.matmul(scores[:, o:o + w],
                                             Bnc[:, 128 * n:128 * (n + 1)],
                                             qk[:, o:o + w], start=False,
                                             stop=(n == NE - 1 and o + w == EP))
                # exp (SCORE_SCALE applied here in fp32)
                nc.scalar.activation(exT[:], scores[:], AF.Exp, scale=SCORE_SCALE)

            # =========== Phase D: transposes, segment sums ===========
            with (
                tc.tile_pool(name="psT", bufs=2, space="PSUM") as ppt,
                tc.tile_pool(name="psD", bufs=1, space="PSUM") as ppd,
                tc.tile_pool(name="psN", bufs=2, space="PSUM") as ppn,
            ):
                for g in range((NCH + 3) // 4):
                    tp = ppt.tile([128, 512], BF16, tag="tp_ex")
                    nchunk = min(4, NCH - 4 * g)
                    for j in range(nchunk):
                        c = 4 * g + j
                        nc.tensor.transpose(tp[:, 128 * j:128 * (j + 1)],
                                            exT[:, 128 * c:128 * (c + 1)], idbf)
                    nc.scalar.activation(exE[:, 512 * g:512 * g + 128 * nchunk],
                                          tp[:, :128 * nchunk], AF.Copy)

                dn = ppd.tile([LA, NE * N_HEADS], F32, tag="dnp")
                for c in range(NCH):
                    nc.tensor.matmul(dn[:], cb("ohEc")[:, LA * c:LA * (c + 1)],
                                     exE[:, 128 * c:128 * (c + 1)],
                                     start=(c == 0), stop=(c == NCH - 1))
                dnS = pw.tile([LA, NE * N_HEADS], F32, tag="dnS")
                nc.vector.tensor_scalar_max(dnS[:], dn[:], 1e-30)
                nc.vector.reciprocal(rden[:], dnS[:])

                NG = 4  # n-groups of 8 for numer accumulation
                for r in range(NG):
                    npt = ppn.tile([LA, 1024], F32, tag="numerp")
                    for c in range(NCH):
                        exv = pw.tile([128, 1024], BF16, tag="exv")
                        # exE slice for n in [8r, 8r+8): free offset 4*8r = 32r
                        src_ex = exE[:, 128 * c + 32 * r:128 * c + 32 * r + 32]
                        src_ex = src_ex.rearrange("p (n h) -> p n h", h=N_HEADS)
                        src_ex = src_ex.unsqueeze(2).to_broadcast((128, 8, HEAD_DIM, N_HEADS))
                        src_v = vE[:, 128 * c:128 * (c + 1)]
                        src_v = src_v.rearrange("p (d h) -> p d h", h=N_HEADS)
                        src_v = src_v.unsqueeze(1).to_broadcast((128, 8, HEAD_DIM, N_HEADS))
                        dst = exv[:].rearrange("p (n d h) -> p n d h",
                                               n=8, d=HEAD_DIM, h=N_HEADS)
                        nc.vector.tensor_tensor(dst, src_ex, src_v, op=ALU.mult)
                        for half in range(2):
                            hs = slice(512 * half, 512 * (half + 1))
                            nc.tensor.matmul(npt[:, hs],
                                             cb("ohEc")[:, LA * c:LA * (c + 1)],
                                             exv[:, hs],
                                             start=(c == 0), stop=(c == NCH - 1))
                    # normalize into attnS
                    npt4 = npt[:].rearrange("p (n d h) -> p n d h",
                                            n=8, d=HEAD_DIM, h=N_HEADS)
                    rd4 = rden[:, 32 * r:32 * r + 32]
                    rd4 = rd4.rearrange("p (n h) -> p n h", h=N_HEADS)
                    rd4 = rd4.unsqueeze(2).to_broadcast((LA, 8, HEAD_DIM, N_HEADS))
                    at4 = attnS[:, 1024 * r:1024 * (r + 1)]
                    at4 = at4.rearrange("p (n d h) -> p n d h",
                                        n=8, d=HEAD_DIM, h=N_HEADS)
                    nc.vector.tensor_tensor(at4, npt4, rd4, op=ALU.mult)

            # =========== Phase E: out-proj ===========
            with tc.tile_pool(name="psO", bufs=2, space="PSUM") as ppo:
                for g in range(4):
                    tp = ppo.tile([128, 512], BF16, tag="tp_o")
                    for j in range(8):
                        n = 8 * g + j
                        nc.tensor.transpose(tp[:, 64 * j:64 * (j + 1)],
                                            attnS[:, 128 * n:128 * (n + 1)],
                                            idbf[0:LA, 0:LA])
                    nc.scalar.activation(oT[:, 512 * g:512 * (g + 1)], tp[:],
                                          AF.Copy)
                outT = pl.tile([128, OUT_COLS], F32, tag="outT")
                for g in range(4):
                    sl = slice(512 * g, 512 * (g + 1))
                    lo1 = ppo.tile([128, 512], F32, tag="o_l1")
                    nc.tensor.matmul(lo1[:], cb("w_o1"), oT[:, sl],
                                     start=True, stop=True)
                    xo = pw.tile([128, 512], BF16, tag="o_x")
                    nc.scalar.activation(xo[:], lo1[:], AF.Silu,
                                         bias=cf("b_o1")[:, 0:1])
                    lo2 = ppo.tile([128, 512], F32, tag="o_l2")
                    nc.tensor.matmul(lo2[:], cb("w_o2"), xo[:],
                                     start=True, stop=True)
                    nc.scalar.activation(outT[:, sl], lo2[:], AF.Identity,
                                         bias=cf("b_o2")[:, 0:1])
                    nc.sync.dma_start(out_d[:, sl], outT[:, sl])

    _split_waits(nc)
    return nc


# ---------------------------------------------------------------------------
# Host side
# ---------------------------------------------------------------------------

_NC_CACHE = None


def _get_nc():
    global _NC_CACHE
    if _NC_CACHE is None:
        _NC_CACHE = build_nc()
    return _NC_CACHE


def _np(x):
    return np.asarray(x)


def _prep_weights(params, e_feat):
    """Shared (non-per-core) pieces of the packed const arrays."""
    z = {}
    def L(mlp, i):
        W, b = mlp[i]
        return _np(W).astype(np.float32), _np(b).astype(np.float32)
    for name, mlp in (("k", params["key_mlp"]), ("v", params["value_mlp"])):
        W1, b1 = L(mlp, 0); W2, b2 = L(mlp, 1); W3, b3 = L(mlp, 2)
        z[f"w_{name}1a"] = W1[:128]
        z[f"w_{name}1b"] = W1[128:128 + KVIN]
        z[f"w_{name}1br"] = W1[128 + KVIN:]
        z[f"b_{name}1"] = b1[:, None]
        z[f"w_{name}2"] = W2
        z[f"b_{name}2"] = b2[:, None]
        z[f"w_{name}3"] = W3
        z[f"b_{name}3"] = b3[:, None]
    # d-outer permutation for v outputs / o1 rows: new row 4*d+h <- old 32*h+d
    l = np.arange(128)
    perm_pos = (l % 32) * 4 + (l // 32)
    inv = np.empty(128, np.int64)
    inv[perm_pos] = l
    W3, b3 = L(params["value_mlp"], 2)
    z["w_v3"] = W3[:, inv]
    z["b_v3"] = b3[inv][:, None]

    Wq1, bq1 = L(params["query_mlp"], 0)
    z["w_q1a"] = Wq1[:128]
    z["efT1"] = np.concatenate([_np(e_feat).astype(np.float32).T,
                                np.ones((1, NE), np.float32)], 0)
    z["w_q1b"] = np.concatenate([Wq1[128:], bq1[None, :]], 0)
    Wq2, bq2 = L(params["query_mlp"], 1)
    Wq3, bq3 = L(params["query_mlp"], 2)
    z["w_q2"] = Wq2
    z["b_q2"] = bq2[:, None]
    z["w_q3"] = Wq3
    bq3blk = np.zeros((128, N_HEADS), np.float32)
    for hh in range(N_HEADS):
        bq3blk[32 * hh:32 * hh + 32, hh] = bq3[32 * hh:32 * hh + 32]
    z["bq3blk"] = bq3blk
    C4 = np.zeros((N_HEADS, 128), np.float32)
    for hh in range(N_HEADS):
        C4[hh, hh::N_HEADS] = 1.0
    z["C4"] = C4

    Wo1, bo1 = L(params["out_proj"], 0)
    Wo2, bo2 = L(params["out_proj"], 1)
    z["w_o1"] = Wo1[inv, :]
    z["b_o1"] = bo1[:, None]
    z["w_o2"] = Wo2
    z["b_o2"] = bo2[:, None]

    centers = np.linspace(0.0, ATT_CUTOFF, RBF_DIM).astype(np.float32)
    z["negC"] = -centers[:, None]
    z["idbf"] = np.eye(128, dtype=np.float32)

    # B_n stationaries, chunked (128, 128*NE): B_n[(h,d), 4n+h] = 1
    Bnc = np.zeros((128, 128 * NE), np.float32)
    hh = np.arange(128) // 32
    for n in range(NE):
        Bnc[np.arange(128), 128 * n + 4 * n + hh] = 1.0
    z["Bnc"] = Bnc.astype(ml_dtypes.bfloat16)
    return z


def _host_prep(inputs):
    h = _np(inputs["h"]).astype(np.float32)
    zt = _np(inputs["z"])
    mask = _np(inputs["mask"]).astype(bool)
    e_feat = _np(inputs["e_feat"]).astype(np.float32)
    src = _np(inputs["att_src"]).astype(np.int64)
    dst = _np(inputs["att_dst"]).astype(np.int64)
    dist = _np(inputs["att_dist"]).astype(np.float32)
    params = inputs["params"]

    B, N, H = h.shape
    flat = B * N
    h_flat = h.reshape(flat, H)
    z_flat = zt.reshape(flat)
    mask_flat = mask.reshape(flat)
    edge_active = mask_flat[src] & mask_flat[dst]
    z_emb = _np(params["z_emb"]).astype(np.float32)

    wz = _prep_weights(params, e_feat)

    # shared packed f32 consts
    cf32 = np.zeros((128, CF32_W), np.float32)
    for name, p, w in CF32_ENTRIES:
        cf32[0:p, CF32_OFF[name]:CF32_OFF[name] + w] = wz[name]

    order = np.argsort(src, kind="stable")
    in_maps = []
    for c in range(NCORES):
        lo, hi = c * LA, (c + 1) * LA
        sel = order[(src[order] >= lo) & (src[order] < hi)]
        ne = sel.shape[0]
        if ne > EP:
            raise ValueError(f"core {c}: {ne} edges exceeds EP={EP}")
        e_src = np.zeros(EP, np.int64)
        e_dst = np.zeros(EP, np.int64)
        e_dist = np.full(EP, 2.5, np.float32)
        e_val = np.zeros(EP, bool)
        e_src[:ne] = src[sel] - lo
        e_dst[:ne] = dst[sel]
        e_dist[:ne] = dist[sel]
        e_val[:ne] = edge_active[sel]

        hdstT = np.ascontiguousarray(h_flat[e_dst].T)
        zr = z_emb[z_flat[e_dst]]
        is_self = ((e_dst == (e_src + lo)) & e_val).astype(np.float32)
        xkvbT = np.concatenate([zr.T, is_self[None, :]], 0).astype(np.float32)

        ohT = np.zeros((LA, EP), np.float32)
        vi = np.nonzero(e_val)[0]
        ohT[e_src[vi], vi] = 1.0
        ohEc = np.ascontiguousarray(
            ohT.reshape(LA, NCH, 128).transpose(2, 1, 0).reshape(128, NCH * LA))

        cbf = np.zeros((128, CBF_W), np.float32)
        core_arrays = dict(wz)
        core_arrays["h_locT"] = h_flat[lo:hi].T
        core_arrays["ohEc"] = ohEc
        for name, p, w in CBF_ENTRIES:
            cbf[0:p, CBF_OFF[name]:CBF_OFF[name] + w] = core_arrays[name]

        m = {
            "d16": np.broadcast_to(e_dist[None, :], (RBF_DIM, EP)).copy(),
            "dE": np.ascontiguousarray(e_dist.reshape(NCH, 128).T),
            "xkvbT": xkvbT.astype(ml_dtypes.bfloat16),
            "hdstT": hdstT.astype(ml_dtypes.bfloat16),
            "cf32": cf32,
            "cbf": cbf.astype(ml_dtypes.bfloat16),
            "ohT": ohT.astype(ml_dtypes.bfloat16),
            "Bnc": wz["Bnc"],
        }
        in_maps.append(m)
    return in_maps


def kernel(**inputs):
    from concourse import bass_utils
    nc = _get_nc()
    in_maps = _host_prep(inputs)
    res = bass_utils.run_bass_kernel_spmd(nc, in_maps, core_ids=list(range(NCORES)))
    outs = []
    for c in range(NCORES):
        outT = np.asarray(res.results[c]["out"])          # (128, 2048) cols=(n,a)
        outs.append(outT.reshape(128, NE, LA).transpose(2, 1, 0))
    full = np.concatenate(outs, 0)                        # (512, 32, 128)
    return np.ascontiguousarray(full.reshape(8, 64, NE, LATENT).astype(np.float32))
